# revision 24
# baseline (speedup 1.0000x reference)
"""MD-LSTM (4-direction 2D LSTM) Trainium2 Bass kernel. v3

Sharding (8 NeuronCores, SPMD): core c handles direction (c % 4) with batch
half (c // 4); the 16-batch half is split into TWO interleaved sub-scans of 8
(A, B).  The two sub-scans are independent recurrences: the tensor engine
runs B's matmuls while A's pointwise tail executes (and vice versa).

Per sub-scan the H,W recurrence runs as anti-diagonal wavefronts: 159 steps,
gates for the valid diagonal cells (<=32) x 8 batch = <=256 matmul columns,
contracting [x(64); 1; 1; h_up(128); h_lf(128)] against [w0; b_hi; b_lo;
u0; u1] (bf16) into PSUM, then the LSTM cell update on ACT/DVE/Pool with
fp32 c-state.

Key scheduling:
- Bias rides the x-projection as two ones-rows (bf16 bias + bf16 residual,
  K=66) so activations need no per-gate bias and the bias is fp32-accurate.
- PSUM: one 2KB bank == one accumulation group.  l,f packed into one bank,
  i,o into another, g alone; only the first matmul per bank has start=True.
- u-matmul order g,l,f,i,o: tanh(g) and sigmoid(l,f) fire while the PE is
  still on i,o; the DVE c-chain starts early.
- pointwise is split: early (tanh_g, sig_lf, sig_io, ig, c-chain) issued
  right after the sub-scan's matmuls; late (tanh_c, h-mult) issued AFTER the
  other sub-scan's matmuls+early block, so the in-order ACT/DVE queues never
  head-of-line-block the other sub-scan's chain.
- h state lives in a 32-slot ring (slot = step % 32) that doubles as the
  output staging buffer: one 16-step chunk DMA out instead of per-step DMAs;
  x is prefetched in 16-step chunks into a 32-slot ring likewise.

State: h (bf16) ring (OC, 32*(8 guard + 256)); c (fp32) double-buffered
(OC, 8 + 256).  Column = guard + y*8 + b; up-neighbor = column offset -8 in
the previous step's slot; guard stays zero; stale slots are never read.

Self-contained: hardcodes all shapes; reads no files.
"""
import numpy as np

import concourse.bass as bass
import concourse.bacc as bacc
import concourse.mybir as mybir
import concourse.tile as tile
from concourse import bass_utils

B, CIN, H, W, OC = 32, 64, 32, 128, 128
CINB = CIN + 2              # +2 ones-rows carrying bias hi/lo
NSTEP = H + W - 1           # 159
BQ = 8                      # batch per sub-scan
SWQ = H * BQ                # 256 max window cells
HWQ = BQ + SWQ              # guard + slots = 264
RING = 32                   # ring slots (2 chunks of CHUNK)
CHUNK = 16                  # steps per DMA chunk
FP = mybir.dt.float32
HF = mybir.dt.float16

# host-side gate reorder: [l, f, i, o, g] (reference order [i, f, g, o, l])
GATE_ORDER = [4, 1, 0, 3, 2]
J_L, J_F, J_I, J_O, J_G = 0, 1, 2, 3, 4


def _window(d):
    return max(0, d - (W - 1)), min(d, H - 1)


def build_kernel():
    nc = bacc.Bacc("TRN2", target_bir_lowering=False, debug=False, num_devices=8)

    xs_d = [nc.dram_tensor(f"x_diag{s}", [CINB, NSTEP * SWQ], HF,
                           kind="ExternalInput") for s in range(2)]
    w0_d = nc.dram_tensor("w0", [CINB, 5 * OC], FP, kind="ExternalInput")
    u0_d = nc.dram_tensor("u0", [OC, 5 * OC], FP, kind="ExternalInput")
    u1_d = nc.dram_tensor("u1", [OC, 5 * OC], FP, kind="ExternalInput")
    outs_d = [nc.dram_tensor(f"out_diag{s}", [OC, NSTEP * HWQ], HF,
                             kind="ExternalOutput") for s in range(2)]

    act = mybir.ActivationFunctionType
    alu = mybir.AluOpType

    with tile.TileContext(nc) as tc:
        with (
            tc.tile_pool(name="const", bufs=1) as cpool,
            tc.tile_pool(name="state", bufs=1) as spool,
            tc.tile_pool(name="gates", bufs=3) as gpool,
            tc.tile_pool(name="tmp", bufs=3) as tpool,
            tc.tile_pool(name="psum", bufs=3, space="PSUM") as ppool,
        ):
            # ---- weights (fp32 load -> bf16 cast once) ----
            w0s = cpool.tile([CINB, 5 * OC], FP, tag="w0")
            u0s = cpool.tile([OC, 5 * OC], FP, tag="u0")
            u1s = cpool.tile([OC, 5 * OC], FP, tag="u1")
            nc.sync.dma_start(w0s[:, :], w0_d.ap())
            nc.sync.dma_start(u0s[:, :], u0_d.ap())
            nc.sync.dma_start(u1s[:, :], u1_d.ap())
            w0m = cpool.tile([CINB, 5 * OC], HF, tag="w0b")
            u0m = cpool.tile([OC, 5 * OC], HF, tag="u0b")
            u1m = cpool.tile([OC, 5 * OC], HF, tag="u1b")
            nc.vector.tensor_copy(w0m[:, :], w0s[:, :])
            nc.vector.tensor_copy(u0m[:, :], u0s[:, :])
            nc.vector.tensor_copy(u1m[:, :], u1s[:, :])

            # ---- rings and state ----
            # x ring: 32 slots of SWQ cols; h ring: 32 slots of HWQ cols
            xr = [spool.tile([CINB, RING * SWQ], HF, tag=f"xr{s}",
                             name=f"xr{s}") for s in range(2)]
            hr = [spool.tile([OC, RING * HWQ], HF, tag=f"hr{s}",
                             name=f"hr{s}") for s in range(2)]
            cb = [[spool.tile([OC, HWQ], HF, tag=f"cb{s}{k}", name=f"cb{s}{k}")
                   for k in range(2)] for s in range(2)]
            for s in range(2):
                nc.vector.memset(hr[s][:, :], 0.0)
                for k in range(2):
                    nc.vector.memset(cb[s][k][:, :], 0.0)

            def load_x_chunk(s, c):
                """DMA x chunk c (steps [16c, 16c+16)) into its ring half."""
                d0 = c * CHUNK
                n = min(CHUNK, NSTEP - d0) * SWQ
                nc.sync.dma_start(
                    xr[s][:, (d0 % RING) * SWQ: (d0 % RING) * SWQ + n],
                    xs_d[s].ap()[:, d0 * SWQ: d0 * SWQ + n])

            def store_h_chunk(s, c):
                """DMA h chunk c (steps [16c, 16c+16)) from its ring half."""
                d0 = c * CHUNK
                n = min(CHUNK, NSTEP - d0) * HWQ
                nc.sync.dma_start(
                    outs_d[s].ap()[:, d0 * HWQ: d0 * HWQ + n],
                    hr[s][:, (d0 % RING) * HWQ: (d0 % RING) * HWQ + n])

            load_x_chunk(0, 0)
            load_x_chunk(1, 0)
            load_x_chunk(0, 1)
            load_x_chunk(1, 1)

            class Step:
                pass

            XORD = (J_L, J_F, J_G, J_I, J_O)
            UORD = (J_L, J_F, J_G, J_I, J_O)

            def mm_x(s, d, st):
                """x-projection matmuls for sub-scan s at step d (h-independent)."""
                y0, y1 = _window(d)
                nwin = (y1 - y0 + 1) * BQ
                st.y0, st.nwin = y0, nwin
                st.lo = BQ + y0 * BQ
                xv = xr[s][:, (d % RING) * SWQ + y0 * BQ:
                           (d % RING) * SWQ + y0 * BQ + nwin]
                # PSUM: one 2KB bank == one accumulation group (start=True
                # zeroes the whole bank).  l,f in one bank, i,o in another,
                # g alone.
                st.ramp = nwin <= 128   # all 4 sigmoid gates fit one bank
                st.lf = ppool.tile([OC, 2 * SWQ], FP, tag="lf", bufs=3,
                                   name=f"lf{s}_{d}")
                st.io = (None if st.ramp else
                         ppool.tile([OC, 2 * SWQ], FP, tag="io", bufs=3,
                                    name=f"io{s}_{d}"))
                st.pg = ppool.tile([OC, SWQ], FP, tag="pg", bufs=1,
                                   name=f"pg{s}_{d}")

                def dst(j):
                    if j == J_G:
                        return st.pg[:, 0:nwin]
                    if st.ramp:
                        return st.lf[:, j * nwin:(j + 1) * nwin]
                    if j in (J_L, J_F):
                        return st.lf[:, (j - J_L) * nwin:(j - J_L + 1) * nwin]
                    return st.io[:, (j - J_I) * nwin:(j - J_I + 1) * nwin]

                st.dst = dst
                starts = (J_L, J_G) if st.ramp else (J_L, J_I, J_G)
                for j in XORD:
                    nc.tensor.matmul(dst(j),
                                     w0m[:, j * OC:(j + 1) * OC], xv,
                                     start=(j in starts), stop=False)

            def mm_u(s, d, st):
                """h-recurrent matmuls for sub-scan s at step d."""
                nwin, lo = st.nwin, st.lo
                pbase = ((d - 1) % RING) * HWQ
                rhs_up = hr[s][:, pbase + lo - BQ: pbase + lo - BQ + nwin]
                rhs_lf = hr[s][:, pbase + lo: pbase + lo + nwin]
                stops = (J_O, J_G) if st.ramp else (J_F, J_O, J_G)
                for j in UORD:
                    nc.tensor.matmul(st.dst(j),
                                     u0m[:, j * OC:(j + 1) * OC], rhs_up,
                                     start=False, stop=False)
                    nc.tensor.matmul(st.dst(j),
                                     u1m[:, j * OC:(j + 1) * OC], rhs_lf,
                                     start=False, stop=(j in stops))

            def dcx_pre(s, d, st, prev):
                """dcx = c_up - c_lf: only needs the previous step's c (Pool,
                off the critical path)."""
                nwin, lo = st.nwin, st.lo
                c_up = cb[s][prev][:, lo - BQ: lo - BQ + nwin]
                c_lf = cb[s][prev][:, lo: lo + nwin]
                dcx = tpool.tile([OC, SWQ], HF, tag=f"dcx{s}", name=f"dcx{s}_{d}")
                nc.gpsimd.tensor_tensor(dcx[:, 0:nwin], c_up, c_lf, alu.subtract)
                st.dcx = dcx

            def pw_early(s, d, st, cur, prev):
                """activations + c-chain; everything except tanh(c) and h."""
                nwin, lo = st.nwin, st.lo
                c_lf = cb[s][prev][:, lo: lo + nwin]
                g4 = gpool.tile([OC, 4 * SWQ], HF, tag=f"g4{s}", name=f"g4{s}_{d}")
                st.g4 = g4
                gg = gpool.tile([OC, SWQ], HF, tag=f"gg{s}", name=f"gg{s}_{d}")
                if st.ramp:
                    # one fused sigmoid over l,f,i,o (single bank)
                    nc.scalar.activation(g4[:, 0:4 * nwin],
                                         st.lf[:, 0:4 * nwin], act.Sigmoid)
                    nc.scalar.activation(gg[:, 0:nwin], st.pg[:, 0:nwin],
                                         act.Tanh)
                else:
                    nc.scalar.activation(g4[:, 0:2 * nwin],
                                         st.lf[:, 0:2 * nwin], act.Sigmoid)
                    nc.scalar.activation(gg[:, 0:nwin], st.pg[:, 0:nwin],
                                         act.Tanh)
                    nc.scalar.activation(g4[:, 2 * nwin:3 * nwin],
                                         st.io[:, 0:nwin], act.Sigmoid)
                l_ = g4[:, 0 * nwin:1 * nwin]
                f_ = g4[:, 1 * nwin:2 * nwin]
                i_ = g4[:, 2 * nwin:3 * nwin]
                # c chain on DVE back-to-back; ig issued AFTER m3 so its
                # wait on sigmoid(i,o) overlaps the chain instead of
                # head-of-line blocking it
                mix = tpool.tile([OC, SWQ], HF, tag=f"mix{s}", name=f"mix{s}_{d}")
                nc.vector.tensor_tensor(mix[:, 0:nwin], l_, st.dcx[:, 0:nwin],
                                        alu.mult)
                nc.vector.tensor_tensor(mix[:, 0:nwin], mix[:, 0:nwin], c_lf,
                                        alu.add)
                nc.vector.tensor_tensor(mix[:, 0:nwin], f_, mix[:, 0:nwin],
                                        alu.mult)
                ig = tpool.tile([OC, SWQ], HF, tag=f"ig{s}", name=f"ig{s}_{d}")
                nc.vector.tensor_tensor(ig[:, 0:nwin], i_, gg[:, 0:nwin], alu.mult)
                cw = cb[s][cur][:, lo: lo + nwin]
                nc.vector.tensor_tensor(cw, mix[:, 0:nwin], ig[:, 0:nwin],
                                        alu.add)
                st.cw = cw

            def pw_late(s, d, st):
                """tanh(c) + h-mult, issued after the OTHER sub-scan's
                matmuls/early block so the in-order ACT/DVE queues don't
                head-of-line-block it."""
                nwin, lo = st.nwin, st.lo
                if st.ramp:
                    o_ = st.g4[:, 3 * nwin:4 * nwin]
                else:
                    ot = tpool.tile([OC, SWQ], HF, tag=f"ot{s}",
                                    name=f"ot{s}_{d}")
                    nc.scalar.activation(ot[:, 0:nwin],
                                         st.io[:, nwin:2 * nwin], act.Sigmoid)
                    o_ = ot[:, 0:nwin]
                th = tpool.tile([OC, SWQ], HF, tag=f"th{s}", name=f"th{s}_{d}")
                nc.scalar.activation(th[:, 0:nwin], st.cw, act.Tanh)
                base = (d % RING) * HWQ
                hwv = hr[s][:, base + lo: base + lo + nwin]
                nc.vector.tensor_tensor(hwv, o_, th[:, 0:nwin], alu.mult)

            scr = ppool.tile([OC, 512], FP, tag="scratch", bufs=1,
                             name="scratch")

            def pe_fill(k):
                """Dummy matmuls on constant tiles: no deps, keep the PE
                clock ramped while it waits for the h chain."""
                for _ in range(k):
                    nc.tensor.matmul(scr[:, 0:512], u1m[:, 0:OC],
                                     u0m[:, 0:512], start=True, stop=True)

            for d in range(NSTEP):
                cur, prev = d % 2, (d + 1) % 2
                stA, stB = Step(), Step()
                mm_x(0, d, stA)
                mm_x(1, d, stB)
                dcx_pre(0, d, stA, prev)
                dcx_pre(1, d, stB, prev)
                pe_fill(3)
                mm_u(1, d, stB)
                pw_early(1, d, stB, cur, prev)
                mm_u(0, d, stA)
                pw_early(0, d, stA, cur, prev)
                pw_late(1, d, stB)
                pw_late(0, d, stA)
                if d % CHUNK == CHUNK - 1:
                    c = d // CHUNK
                    # h chunk done -> store; prefetch x chunk c+2
                    store_h_chunk(0, c)
                    store_h_chunk(1, c)
                    if (c + 2) * CHUNK < NSTEP:
                        load_x_chunk(0, c + 2)
                        load_x_chunk(1, c + 2)
            # tail: steps 144..158 are chunk 9 (15 steps)
            store_h_chunk(0, NSTEP // CHUNK)
            store_h_chunk(1, NSTEP // CHUNK)

    nc.compile()
    return nc


_NC_CACHE = {}


def _get_nc():
    if "nc" not in _NC_CACHE:
        _NC_CACHE["nc"] = build_kernel()
    return _NC_CACHE["nc"]


def _flip(x, d):
    if d == 1:
        return x[:, :, :, ::-1]
    if d == 2:
        return x[:, :, ::-1, :]
    if d == 3:
        return x[:, :, ::-1, ::-1]
    return x


def _make_x_diag(x_nat):
    """(CIN,H,W,BQ) -> (CINB, NSTEP*SWQ) diagonal layout with ones-rows."""
    arr = np.zeros((CINB, NSTEP, H, BQ), np.float32)
    for y in range(H):
        arr[:CIN, y:y + W, y, :] = x_nat[:, y, :, :]
    arr[CIN:, :, :, :] = 1.0
    return arr.reshape(CINB, NSTEP * SWQ)


def _decode(out_diag):
    """(OC, NSTEP*HWQ) fp32 -> (BQ, OC, H, W); skip the guard columns."""
    arr = out_diag.reshape(OC, NSTEP, HWQ)[:, :, BQ:].reshape(OC, NSTEP, H, BQ)
    out = np.empty((BQ, OC, H, W), np.float32)
    for y in range(H):
        out[:, :, y, :] = arr[:, y:y + W, y, :].transpose(2, 0, 1)
    return out


def kernel(x, w0, u0, u1, b, trace=False, _res=[None]):
    x = np.asarray(x, np.float32)
    w0 = np.asarray(w0, np.float32)
    u0 = np.asarray(u0, np.float32)
    u1 = np.asarray(u1, np.float32)
    b = np.asarray(b, np.float32)

    perm = np.concatenate([np.arange(g * OC, (g + 1) * OC) for g in GATE_ORDER])
    in_maps = []
    for c in range(8):
        dirn, half = c % 4, c // 4
        xs = _flip(x[half * 16:(half + 1) * 16], dirn)          # (16,CIN,H,W)
        x_nat = np.ascontiguousarray(xs.transpose(1, 2, 3, 0))  # (CIN,H,W,16)
        # bias split into bf16 hi + residual lo rows so it lands fp32-accurate
        bp = b[dirn][perm]
        b_hi = bp.astype(np.float16).astype(np.float32)
        b_lo = bp - b_hi
        w0b = np.concatenate([w0[dirn][:, perm], b_hi[None, :], b_lo[None, :]],
                             axis=0)
        m = {
            "w0": np.ascontiguousarray(w0b),
            "u0": np.ascontiguousarray(u0[dirn][:, perm]),
            "u1": np.ascontiguousarray(u1[dirn][:, perm]),
        }
        for s in range(2):
            m[f"x_diag{s}"] = _make_x_diag(
                x_nat[:, :, :, s * BQ:(s + 1) * BQ]).astype(np.float16)
        in_maps.append(m)

    nc = _get_nc()
    res = bass_utils.run_bass_kernel_spmd(nc, in_maps, list(range(8)), trace=trace)
    _res[0] = res

    out = np.empty((B, 4, OC, H, W), np.float32)
    for c in range(8):
        dirn, half = c % 4, c // 4
        for s in range(2):
            od = np.asarray(res.results[c][f"out_diag{s}"]).astype(np.float32)
            lo = half * 16 + s * BQ
            out[lo:lo + BQ, dirn] = _decode(od)
    return out
